# revision 1
# baseline (speedup 1.0000x reference)
"""Multi-head self-attention (B=8, S=1024, D=1024, H=16) on 8 TRN2 NeuronCores.

Sharding: data-parallel over batch — one batch element per core, weights
replicated; no collectives needed.

v4 = v2 (all-bf16 matmuls) + v3 (PV with P^T stationary / 65-wide moving V,
per-partition softmax normalize, PE transpose back to attnt layout) + group
-level software pipelining: the attention inner loop is ACT-bound (exp of
16.8M scores ~= 1us/step vs ~0.65us of PE work), so the next group's QKV
projection chains — and, in the last group, the first output-projection
chains — are emitted interleaved into the attention steps. The in-order PE
stream then always has a big matmul chain to chew while ACT catches up.
End-to-end error vs the fp32 reference: ~5e-3 (gate 2e-2).
"""

from contextlib import ExitStack

import numpy as np

import concourse.mybir as mybir
import concourse.tile as tile
from concourse import bacc
from concourse.bass_utils import run_bass_kernel_spmd
from concourse.masks import make_identity

S = 1024  # sequence length (per core batch element)
D = 1024  # embed dim
H = 16  # heads
HD = 64  # head dim
P = 128  # partitions
NCORES = 8
NG = 4  # head groups (4 heads / 256 channels each)
GC = 256  # channels per group
SCALE = 1.0 / 8.0  # 1/sqrt(HD)

F32 = mybir.dt.float32
F32R = mybir.dt.float32r
BF16 = mybir.dt.bfloat16
AF = mybir.ActivationFunctionType


def make_pools(ctx, tc):
    return {
        "const": ctx.enter_context(tc.tile_pool(name="const", bufs=1)),
        "xtp": ctx.enter_context(tc.tile_pool(name="xtp", bufs=1)),
        "xinp": ctx.enter_context(tc.tile_pool(name="xinp", bufs=3)),
        "wblkp": ctx.enter_context(tc.tile_pool(name="wblkp", bufs=6)),
        "qkp": ctx.enter_context(tc.tile_pool(name="qkp", bufs=2)),
        "vgp": ctx.enter_context(tc.tile_pool(name="vgp", bufs=2)),
        "ptp": ctx.enter_context(tc.tile_pool(name="ptp", bufs=2)),
        "wpp": ctx.enter_context(tc.tile_pool(name="wpp", bufs=1)),
        "smp": ctx.enter_context(tc.tile_pool(name="smp", bufs=4)),
        "ps": ctx.enter_context(tc.tile_pool(name="ps", bufs=2, space="PSUM")),
    }


def emit_mha(pools, tc, out, x, wqkv, bqkv, wproj, bproj):
    nc = tc.nc

    const = pools["const"]
    xt_pool = pools["xtp"]
    xin_pool = pools["xinp"]
    wblk_pool = pools["wblkp"]
    qk_pool = pools["qkp"]
    vg_pool = pools["vgp"]
    pt_pool = pools["ptp"]
    wp_pool = pools["wpp"]
    sm_pool = pools["smp"]
    ps = pools["ps"]

    # ---- X^T arrives pre-transposed from the host: DMA straight into the
    # xth tiles, first-needed half first, so QKV(0) starts ~3us in ----
    xth = [
        xt_pool.tile([P, 8, S // 2], BF16, tag=f"xt{h}", name=f"xt{h}")
        for h in range(2)
    ]
    # halves on different queues so both X^T transfers run in parallel
    nc.sync.dma_start(xth[0], x[0])
    nc.scalar.dma_start(xth[1], x[1])

    # ---- constants / biases ----
    identf = const.tile([P, P], F32, name="identf")
    make_identity(nc, identf)
    ident = const.tile([P, P], BF16, name="ident")
    nc.vector.tensor_copy(ident, identf)
    onesf = const.tile([P, P], F32, name="onesf")
    nc.vector.memset(onesf, 1.0)
    # dummy exp: pulls the ACT table load off the first real exp's
    # critical path (ACT is idle until ~12us otherwise)
    expwarm = const.tile([1, 8], BF16, name="expwarm")
    nc.scalar.activation(expwarm, onesf[0:1, 0:8], AF.Exp)
    ones128 = const.tile([1, P], F32R, name="ones128")
    nc.vector.tensor_copy(ones128, onesf[0:1, :])

    # b_qkv striped per-partition: b_sb[p, col] = b_qkv[col*128 + p]
    b_sb = const.tile([P, 24], F32, name="b_sb")
    nc.sync.dma_start(b_sb, bqkv.rearrange("(col p) -> p col", p=P))
    bq_s = const.tile([P, 8], F32, name="bq_s")  # pre-scaled Q bias
    nc.vector.tensor_scalar_mul(bq_s, b_sb[:, 0:8], SCALE)

    # V and proj biases broadcast to [128, D] via ones-matmul
    def bias_broadcast(row, dst):
        for ch in range(2):
            psb = ps.tile([P, 512], F32, tag="mm", bufs=2, name="psb")
            nc.tensor.matmul(
                psb, lhsT=ones128, rhs=row[:, ch * 512 : (ch + 1) * 512],
                start=True, stop=True,
            )
            nc.vector.tensor_copy(dst[:, ch * 512 : (ch + 1) * 512], psb)

    bvrow = xin_pool.tile([1, D], F32R, tag="bias", name="bvrow")
    nc.gpsimd.dma_start(bvrow, bqkv[2 * D : 3 * D].rearrange("(a c) -> a c", a=1))
    bvb = const.tile([P, D], F32, name="bvb")
    bias_broadcast(bvrow, bvb)

    def xt_slice(ko, s0, s1):
        # contiguous [s0:s1) slice of X^T row-block ko; must stay in one half
        h = s0 // 512
        assert (s1 - 1) // 512 == h
        return xth[h][:, ko, s0 - h * 512 : s1 - h * 512]

    attnt = xt_pool.tile([P, 8, S], BF16, tag="attnt", name="attnt")

    # ---- W DMA emission for one group (bf16 straight from HBM) ----
    def emit_w_group(g):
        wq = wblk_pool.tile([P, 8, GC], BF16, tag="wblk", name="wq")
        wk = wblk_pool.tile([P, 8, GC], BF16, tag="wblk", name="wk")
        wv = wblk_pool.tile([P, 8, GC], BF16, tag="wblk", name="wv")
        # K streams first: the QKV prefix consumes wk before wq, wv last
        for t, wdst in ((1, wk), (0, wq), (2, wv)):
            nc.gpsimd.dma_start(wdst, wqkv[g, t])
        return wq, wk, wv

    # ---- QKV projection chains for one group, as deferred units ----
    def qkv_units(g, wq, wk, wv):
        qt = qk_pool.tile([P, 2, S], BF16, tag="qt", name="qt")
        kt = qk_pool.tile([P, 2, S], BF16, tag="kt", name="kt")
        vg = vg_pool.tile([P, 8, 4, HD + 1], BF16, tag="vg", name="vg")
        units = []

        def q_unit(cb, qch):
            def emit():
                sl = slice(qch * 512, (qch + 1) * 512)
                psq = ps.tile([P, 512], F32, tag="mm", bufs=2, name="psq")
                for ko in range(8):
                    nc.tensor.matmul(
                        psq,
                        lhsT=wq[:, ko, cb * P : (cb + 1) * P],
                        rhs=xt_slice(ko, qch * 512, (qch + 1) * 512),
                        start=(ko == 0),
                        stop=(ko == 7),
                    )
                nc.vector.tensor_scalar(
                    qt[:, cb, sl], psq,
                    SCALE, bq_s[:, 2 * g + cb : 2 * g + cb + 1],
                    mybir.AluOpType.mult, mybir.AluOpType.add,
                )
            return emit

        def k_unit(cb, qch):
            def emit():
                sl = slice(qch * 512, (qch + 1) * 512)
                psk = ps.tile([P, 512], F32, tag="mm", bufs=2, name="psk")
                for ko in range(8):
                    nc.tensor.matmul(
                        psk,
                        lhsT=wk[:, ko, cb * P : (cb + 1) * P],
                        rhs=xt_slice(ko, qch * 512, (qch + 1) * 512),
                        start=(ko == 0),
                        stop=(ko == 7),
                    )
                nc.vector.tensor_scalar(
                    kt[:, cb, sl], psk,
                    b_sb[:, 8 + 2 * g + cb : 8 + 2 * g + cb + 1], None,
                    mybir.AluOpType.add,
                )
            return emit

        def vones_unit():
            def emit():
                nc.vector.tensor_copy(
                    vg[:, :, :, HD],
                    onesf[:, 0:32].rearrange("p (a b) -> p a b", a=8),
                )
            return emit

        def v_unit(so):
            def emit():
                psv = ps.tile([P, GC], F32, tag="mm", bufs=2, name="psv")
                for ko in range(8):
                    nc.tensor.matmul(
                        psv,
                        lhsT=xt_slice(ko, so * P, (so + 1) * P),
                        rhs=wv[:, ko, :],
                        start=(ko == 0),
                        stop=(ko == 7),
                    )
                nc.vector.tensor_add(
                    out=vg[:, so, :, 0:HD],
                    in0=psv.rearrange("p (h c) -> p h c", h=4),
                    in1=bvb[:, g * GC : (g + 1) * GC].rearrange(
                        "p (h c) -> p h c", h=4
                    ),
                )
            return emit

        units.append(vones_unit())
        # K first (scores need K and Q; V needed only after first exp)
        for cb in range(2):
            for qch in range(2):
                units.append(k_unit(cb, qch))
                units.append(q_unit(cb, qch))
        for so in range(8):
            units.append(v_unit(so))
        return qt, kt, vg, units

    # ---- output projection chains as deferred units ----
    wp_tiles = {}

    def emit_wp(ch):
        sl = slice(ch * 512, (ch + 1) * 512)
        wp = wp_pool.tile([P, 8, 512], BF16, tag="wp", bufs=2, name="wp")
        nc.gpsimd.dma_start(wp, wproj[ch])
        wp_tiles[ch] = wp

    bpb = const.tile([P, D], F32, name="bpb")

    def proj_unit(ch, so, force_mm=False):
        def emit():
            sl = slice(ch * 512, (ch + 1) * 512)
            wp = wp_tiles[ch]
            if force_mm or so % 2 == 0:
                psp = ps.tile([P, 512], F32, tag="mm", bufs=2, name="psp")
            else:
                psp = ps.tile([P, 512], F32, tag="sc", bufs=2, name="pspw")
            for ko in range(8):
                nc.tensor.matmul(
                    psp,
                    lhsT=attnt[:, ko, so * P : (so + 1) * P],
                    rhs=wp[:, ko, :],
                    start=(ko == 0),
                    stop=(ko == 7),
                )
            ot = sm_pool.tile([P, 512], F32, tag="ot", bufs=3, name="ot")
            nc.vector.tensor_add(out=ot, in0=psp, in1=bpb[:, sl])
            nc.sync.dma_start(out[so * P : (so + 1) * P, sl], ot)
        return emit

    # ---- attention for one group, pumping deferred units into the
    # ACT-bound inner loop; pump_sched maps (qch, pp) -> units to emit
    # interleaved in that quadrant (None key = spread over all) ----
    def emit_attention(g, qt, kt, vg, pump_sched, rates=None):
        spread = iter(pump_sched.get(None, ()))
        quad = [iter(())]
        rate = [0.5]
        credit = [0.0]

        def pump():
            credit[0] += rate[0]
            while credit[0] >= 1.0:
                credit[0] -= 1.0
                u = next(quad[0], None)
                if u is None:
                    u = next(spread, None)
                if u is None:
                    credit[0] = 0.0
                    return
                u()

        for qch in range(2):
            qsl = slice(qch * 512, (qch + 1) * 512)
            for pp in range(2):
                quad[0] = iter(pump_sched.get((qch, pp), ()))
                rate[0] = (rates or {}).get((qch, pp), 0.5)
                credit[0] = 0.0
                heads = (2 * pp, 2 * pp + 1)  # even, odd within group
                pnums = [
                    ps.tile([P, 4, HD + 1], F32, tag="pv", bufs=2, name=f"pnum{i}")
                    for i in range(2)
                ]
                prev = None
                for ko in range(8):
                    scs, pts = [], []
                    for i, hb in enumerate(heads):
                        scs.append(
                            ps.tile([P, 512], F32, tag="sc", bufs=2, name="pssc")
                        )
                        pts.append(
                            pt_pool.tile([P, 512], BF16, tag="pt", bufs=4, name="pt")
                        )
                        poff = (hb % 2) * HD
                        nc.tensor.matmul(
                            scs[i],
                            lhsT=kt[poff : poff + HD, pp, ko * P : (ko + 1) * P],
                            rhs=qt[poff : poff + HD, pp, qsl],
                            start=True,
                            stop=True,
                        )
                    for i in range(2):
                        nc.scalar.activation(pts[i], scs[i], AF.Exp)
                    pump()
                    if prev is not None:
                        pko, ppts = prev
                        for i in range(2):
                            for qb in range(4):
                                # the whole [P, 4, 65] tile lives in one PSUM
                                # zero region: start once (zeroing the bank),
                                # stop once at the last write
                                nc.tensor.matmul(
                                    pnums[i][:, qb],
                                    lhsT=ppts[i][:, qb * P : (qb + 1) * P],
                                    rhs=vg[:, pko, heads[i]],
                                    start=(pko == 0 and qb == 0),
                                    stop=(pko == 7 and qb == 3),
                                )
                    prev = (ko, pts)
                pko, ppts = prev
                for i in range(2):
                    for qb in range(4):
                        nc.tensor.matmul(
                            pnums[i][:, qb],
                            lhsT=ppts[i][:, qb * P : (qb + 1) * P],
                            rhs=vg[:, pko, heads[i]],
                            start=(pko == 0 and qb == 0),
                            stop=(pko == 7 and qb == 3),
                        )
                for i, hb in enumerate(heads):
                    poff = (hb % 2) * HD
                    rec = sm_pool.tile([P, 4], F32, tag="rec", bufs=2, name="rec")
                    nc.vector.reciprocal(rec, pnums[i][:, :, HD])
                    atq = sm_pool.tile([P, 4, HD], BF16, tag="atq", bufs=2, name="atq")
                    for qb in range(4):
                        nc.vector.tensor_scalar(
                            atq[:, qb], pnums[i][:, qb, 0:HD],
                            rec[:, qb : qb + 1], None,
                            mybir.AluOpType.mult,
                        )
                    for qb in range(4):
                        pstt = ps.tile([HD, P], BF16, tag="tb", bufs=2, name="pstt")
                        nc.tensor.transpose(pstt, atq[:, qb], ident)
                        nc.vector.tensor_copy(
                            attnt[
                                poff : poff + HD, 2 * g + pp,
                                qch * 512 + qb * P : qch * 512 + (qb + 1) * P,
                            ],
                            pstt,
                        )
        # drain any unpumped units
        while True:
            u = next(quad[0], None)
            if u is None:
                u = next(spread, None)
            if u is None:
                break
            u()

    # ---- main pipeline ----
    wq, wk, wv = emit_w_group(0)
    qt, kt, vg, units0 = qkv_units(0, wq, wk, wv)
    # units0 order: [vones, k00, q00, k01, q01, k10, q10, k11, q11, v0..v7]
    # scores for a quadrant span ALL key positions -> need both K halves of
    # that pp; only the Q half is quadrant-local
    for u in (units0[0], units0[1], units0[3], units0[2]):  # vones, k00, k01, q00
        u()
    g0_sched = {
        (0, 0): units0[9:17] + [units0[5], units0[7], units0[6]],
        (0, 1): [units0[4], units0[8]],
    }
    g0_rates = {(0, 0): 2}

    g3_vunits = []
    for g in range(4):
        if g < 3:
            wn = emit_w_group(g + 1)
            qtn, ktn, vgn, units = qkv_units(g + 1, *wn)
            if g == 2:
                # hold group 3's V chains back: attention(3)'s first quadrant
                # otherwise has nothing to pump and idles PE behind exp
                g3_vunits[:] = units[9:17]
                units = units[0:9]
            pump_sched = {None: units}
            if g == 0:
                pump_sched.update(g0_sched)
        else:
            # last group: prefetch w_proj + proj bias once the attention
            # steps are underway (no attnt dependency in the prefetch)
            def _wp_unit():
                emit_wp(0)
                emit_wp(1)
                bprow = xin_pool.tile([1, D], F32R, tag="bias", name="bprow")
                nc.gpsimd.dma_start(
                    bprow, bproj.rearrange("(a c) -> a c", a=1)
                )
                bias_broadcast(bprow, bpb)

            # the first-half (s < 512) proj chains' attnt inputs are fully
            # written once both (qch=0, pp) iterations retire; pump the
            # mm-bank ones into the ACT-bound qch=1 steps
            pump_sched = {
                (0, 0): list(g3_vunits),
                (0, 1): [_wp_unit],
                (1, 0): [proj_unit(0, so, True) for so in range(4)],
                (1, 1): [proj_unit(1, so, True) for so in range(4)],
            }
        rates = g0_rates if g == 0 else ({(0, 0): 1, (1, 0): 1, (1, 1): 1} if g == 3 else None)
        emit_attention(g, qt, kt, vg, pump_sched, rates=rates)
        if g < 3:
            qt, kt, vg = qtn, ktn, vgn

    # output projection tail (s-rows 512:1024)
    for ch in range(2):
        for so in range(4, 8):
            proj_unit(ch, so)()


def build_nc(repeat=1):
    nc = bacc.Bacc("TRN2", target_bir_lowering=False, debug=False, num_devices=NCORES)
    # packed host layouts: every DMA below is a contiguous block per
    # partition (no striding), shaped exactly like its SBUF tile
    x = nc.dram_tensor("query", [2, P, 8, S // 2], BF16, kind="ExternalInput").ap()
    wqkv = nc.dram_tensor(
        "w_qkv", [NG, 3, P, 8, GC], BF16, kind="ExternalInput"
    ).ap()
    bqkv = nc.dram_tensor("b_qkv", [3 * D], F32, kind="ExternalInput").ap()
    wproj = nc.dram_tensor("w_proj", [2, P, 8, 512], BF16, kind="ExternalInput").ap()
    bproj = nc.dram_tensor("b_proj", [D], F32, kind="ExternalInput").ap()
    out = nc.dram_tensor("out", [S, D], F32, kind="ExternalOutput").ap()
    with (
        tile.TileContext(nc) as tc,
        ExitStack() as ctx,
        nc.allow_low_precision(reason="bf16 matmul pipeline (~5e-3)"),
    ):
        pools = make_pools(ctx, tc)
        for _ in range(repeat):
            emit_mha(pools, tc, out, x, wqkv, bqkv, wproj, bproj)
    nc.compile()
    return nc


_NC_CACHE = None


def _get_nc():
    global _NC_CACHE
    if _NC_CACHE is None:
        _NC_CACHE = build_nc()
    return _NC_CACHE


def make_in_maps(query, w_qkv, b_qkv, w_proj, b_proj):
    import ml_dtypes

    bf = ml_dtypes.bfloat16
    f = np.float32
    # w_qkv [D, 3D] -> [group, tensor(q/k/v), p, ko, c]
    wq = np.asarray(w_qkv, dtype=f).astype(bf)
    wq = wq.reshape(8, P, 3, NG, GC)  # [ko, p, tensor, group, c]
    wq = np.ascontiguousarray(wq.transpose(3, 2, 1, 0, 4))  # [g, t, p, ko, c]
    # w_proj [D, D] -> [ch, p, ko, s]
    wp = np.asarray(w_proj, dtype=f).astype(bf)
    wp = wp.reshape(8, P, 2, 512)  # [ko, p, ch, s]
    wp = np.ascontiguousarray(wp.transpose(2, 1, 0, 3))
    shared = {
        "w_qkv": wq,
        "b_qkv": np.ascontiguousarray(b_qkv, dtype=f),
        "w_proj": wp,
        "b_proj": np.ascontiguousarray(b_proj, dtype=f),
    }
    # query [B, S, D] -> per core X^T packed [h, p, ko, s_half]
    qbf = np.asarray(query, dtype=f).astype(bf)
    maps = []
    for i in range(NCORES):
        xt = qbf[i].T  # [D, S] = [(ko p), s]
        xt = xt.reshape(8, P, 2, 512)  # [ko, p, h, s]
        maps.append(
            {"query": np.ascontiguousarray(xt.transpose(2, 1, 0, 3)), **shared}
        )
    return maps


def kernel(query, w_qkv, b_qkv, w_proj, b_proj):
    nc = _get_nc()
    in_maps = make_in_maps(query, w_qkv, b_qkv, w_proj, b_proj)
    res = run_bass_kernel_spmd(nc, in_maps, core_ids=list(range(NCORES)))
    return np.stack([res.results[i]["out"] for i in range(NCORES)]).astype(np.float32)

